# revision 21
# baseline (speedup 1.0000x reference)
"""MLA (absorbed-weight multi-head latent attention) TRN2 Bass kernel.

Problem: B=2, N=NKV=2048, E=4096, H=16, HD=256, LQ=512, LKV=256.
  C_q  = Q @ Wq_d                 [B,N,LQ]
  C_kv = K @ Wkv_d                [B,Nkv,LKV]
  CqWqk = (C_q @ W_qk)            [B,N,H,LKV]
  scores = einsum('bnhl,bkl->bhnk', CqWqk, C_kv) / sqrt(LKV)
  attn = softmax(scores, -1)
  V_up = (C_kv @ Wv_u)            [B,Nkv,H,HD]
  out  = einsum('bhnk,bkhd->bnhd', attn, V_up) -> [B,N,E]

Sharding: 8 cores = (batch b in 0..1) x (query quarter q in 0..3).
Each core handles n-rows [q*512,(q+1)*512) of batch b for ALL heads.

Strategy vs f32r baseline:
- fp8e4m3 DoubleRow matmuls (contract 256/pass at 0.5 cyc/row = 4x f32r)
  with HI+LO error compensation on the score path: every score-path
  tensor X is stored as fp8(X) + fp8(X - fp8(X)); products use 3 terms
  (hh + hl + lh), cutting score error from ~4% to ~0.1% while keeping
  most of the fp8 speed. (Single-fp8 scores fail the 2% gate: score
  error lands ~1:1 in the output via softmax-value correlation.)
- Latent-space PV: out = (P @ C_kv) @ Wv_u instead of P @ (C_kv@Wv_u),
  eliminating the per-head V_up projection; PV uses 2 terms
  (C_kv_hi + C_kv_lo) x P_fp8.
- Softmax denominator via tiny ones-rhs matmuls -> [128,1] per n-chunk
  (single start=True then accumulate: start clears the whole PSUM bank).
- exp on Act engine reads 2 PSUM banks/instruction, writes fp8 direct,
  with bias -3.5 to keep exp within fp8 range (max observed score 7.6).
- kc loop software-pipelined (S one pair ahead of PV/D); CqWqk^T for
  head h+1 and the tail of head h-1 (up-proj fp16 + normalize) run
  inside head h's Act-bound window, each in the single psC bank.
- Host pre-quantizes inputs to fp8 hi/lo pairs; weights prescaled
  (x64/x16) so device values are ~N(0,1). Output in fp16.

PSUM (8 banks): psA0+psA1 [128,2,512] (4): C_q accum -> S ring (ping-
pong by pair parity; separate tiles because dependency tracking is
tensor-granular). psB [128,2,512] (2): C_kv kslice accum -> PV accum.
psC [128,512] (1): CqWqk / transposes / up-proj, rotating. psD (1).
"""
import numpy as np
import ml_dtypes

B, N, NKV, E, H = 2, 2048, 2048, 4096, 16
HD, LQ, LKV = 256, 512, 256
NCORES = 8
NQ = N // 4          # 512 query rows per core
ECH = E // 128       # 32 e-chunks
EPAIR = ECH // 2     # 16 e-pairs (DoubleRow contracts 256)
KCH = NKV // 128     # 16 k-chunks
KPAIR = KCH // 2     # 8 k-pairs == 8 kslices of 256
KSL = 8
NCK = NQ // 128      # 4 n-chunks per core

_cache = {}


def build_nc(iters=1, stop_after="full"):
    import concourse.bass as bass
    from concourse import bacc
    import concourse.mybir as mybir
    import concourse.tile as tile

    dt = mybir.dt
    f32 = dt.float32
    f8 = dt.float8e4
    f16 = dt.float16
    Exp = mybir.ActivationFunctionType.Exp
    DR = mybir.MatmulPerfMode.DoubleRow
    Sub = mybir.AluOpType.subtract
    Mult = mybir.AluOpType.mult
    do_heads = stop_after == "full"

    nc = bacc.Bacc(None, target_bir_lowering=False)
    QTH = nc.dram_tensor("QTH", [E, NQ], f8, kind="ExternalInput")
    QTL = nc.dram_tensor("QTL", [E, NQ], f8, kind="ExternalInput")
    KTH = nc.dram_tensor("KTH", [E, NKV], f8, kind="ExternalInput")
    KTL = nc.dram_tensor("KTL", [E, NKV], f8, kind="ExternalInput")
    WQDH = nc.dram_tensor("WQDH", [E, LQ], f8, kind="ExternalInput")
    WQDL = nc.dram_tensor("WQDL", [E, LQ], f8, kind="ExternalInput")
    WQKH = nc.dram_tensor("WQKH", [LQ, H * LKV], f8, kind="ExternalInput")
    WQKL = nc.dram_tensor("WQKL", [LQ, H * LKV], f8, kind="ExternalInput")
    WKVDH = nc.dram_tensor("WKVDH", [E, LKV], f8, kind="ExternalInput")
    WKVDL = nc.dram_tensor("WKVDL", [E, LKV], f8, kind="ExternalInput")
    WVU = nc.dram_tensor("WVU", [LKV, H * HD], f16, kind="ExternalInput")
    IDN = nc.dram_tensor("IDN", [128, 128], f8, kind="ExternalInput")
    OUT = nc.dram_tensor("OUT", [NQ, E], f16, kind="ExternalOutput")

    with tile.TileContext(nc) as tc:
        with tc.tile_pool(name="persist", bufs=1) as persist, \
             tc.tile_pool(name="psAp", bufs=1, space="PSUM") as psAp, \
             tc.tile_pool(name="psBp", bufs=1, space="PSUM") as psBp, \
             tc.tile_pool(name="psCp", bufs=1, space="PSUM") as psCp, \
             tc.tile_pool(name="psDp", bufs=1, space="PSUM") as psDp, \
             tc.tile_pool(name="qwp", bufs=2) as qwp, \
             tc.tile_pool(name="ktp", bufs=2) as ktp, \
             tc.tile_pool(name="cqwp", bufs=2) as cqwp, \
             tc.tile_pool(name="hp", bufs=2) as hp, \
             tc.tile_pool(name="ptp", bufs=5) as ptp:
            loop_ctx = tc.For_i(0, iters, 1,
                                hint_engines=(mybir.EngineType.PE,)) \
                if iters > 1 else None
            if loop_ctx is not None:
                loop_ctx.__enter__()

            psA0 = psAp.tile([128, 2, 512], f32, name="psA0")
            psA1 = psAp.tile([128, 2, 512], f32, name="psA1")
            psB = psBp.tile([128, 2, 512], f32)
            psD = psDp.tile([128, 4], f32)

            cqth = persist.tile([128, 4, NQ], f8)
            cqtl = persist.tile([128, 4, NQ], f8)
            ckvth = [persist.tile([128, 2, 256], f8, name=f"ckvth{k}")
                     for k in range(KSL)]
            ckvtl = [persist.tile([128, 2, 256], f8, name=f"ckvtl{k}")
                     for k in range(KSL)]
            ckv8h = [persist.tile([128, 2, LKV], f8, name=f"ckv8h{k}")
                     for k in range(KSL)]
            ckv8l = [persist.tile([128, 2, LKV], f8, name=f"ckv8l{k}")
                     for k in range(KSL)]
            ones8 = persist.tile([128, 2, 1], f8)
            idn8 = persist.tile([128, 128], f8)
            bias2 = persist.tile([128, 1], f32)
            wkvdh = persist.tile([128, ECH, LKV], f8)
            wkvdl = persist.tile([128, ECH, LKV], f8)
            wqkh = persist.tile([128, 4, H * LKV], f8)
            wqkl = persist.tile([128, 4, H * LKV], f8)
            wvu16 = hp.tile([128, 2, H * HD], f16, tag="wvu")

            nc.vector.memset(bias2, -3.5)
            nc.vector.memset(ones8, 1.0)
            nc.sync.dma_start(out=idn8, in_=IDN[:, :])

            # qt/wqd streamed in chunks of 2 e-pairs
            def load_qw(c):
                tiles = {}
                for nm, src in (("qh", QTH), ("ql", QTL),
                                ("wh", WQDH), ("wl", WQDL)):
                    t = qwp.tile([128, 4, 512], f8, tag=nm)
                    nc.sync.dma_start(
                        out=t,
                        in_=src.rearrange("(c p) x -> p c x", p=128)
                        [:, 4 * c:4 * (c + 1), :])
                    tiles[nm] = t
                return tiles

            nc.sync.dma_start(
                out=wkvdh, in_=WKVDH.rearrange("(c p) l -> p c l", p=128))
            nc.sync.dma_start(
                out=wkvdl, in_=WKVDL.rearrange("(c p) l -> p c l", p=128))

            def load_kt(ks):
                th = ktp.tile([128, ECH, 256], f8, tag="kth")
                tl = ktp.tile([128, ECH, 256], f8, tag="ktl")
                nc.sync.dma_start(
                    out=th, in_=KTH[:, ks * 256:(ks + 1) * 256]
                    .rearrange("(c p) n -> p c n", p=128))
                nc.sync.dma_start(
                    out=tl, in_=KTL[:, ks * 256:(ks + 1) * 256]
                    .rearrange("(c p) n -> p c n", p=128))
                return th, tl

            kt_tiles = [load_kt(0), load_kt(1)]
            nc.sync.dma_start(out=wqkh,
                              in_=WQKH.rearrange("(d p) m -> p d m", p=128))
            nc.sync.dma_start(out=wqkl,
                              in_=WQKL.rearrange("(d p) m -> p d m", p=128))
            for ks in range(2, KSL):
                kt_tiles.append(load_kt(ks))
            nc.sync.dma_start(out=wvu16,
                              in_=WVU.rearrange("(l p) e -> p l e", p=128))

            # ---------- phase 1: C_q^T (3-term hi/lo) ----------
            for c in range(8):
                t = load_qw(c)
                for jj in range(2):
                    j = 2 * c + jj
                    sl = slice(2 * jj, 2 * jj + 2)
                    for lc in range(4):
                        ps = (psA0 if lc < 2 else psA1)[:, lc % 2, :]
                        msl = slice(lc * 128, (lc + 1) * 128)
                        for ti, (wt, qt) in enumerate(
                                ((t["wh"], t["qh"]), (t["wh"], t["ql"]),
                                 (t["wl"], t["qh"]))):
                            nc.tensor.matmul(
                                ps, wt[:, sl, msl], qt[:, sl, :],
                                start=(j == 0 and ti == 0),
                                stop=(j == EPAIR - 1 and ti == 2),
                                perf_mode=DR)
            for lc in range(4):
                ps = (psA0 if lc < 2 else psA1)[:, lc % 2, :]
                nc.vector.tensor_scalar_mul(cqth[:, lc, :], ps, 1.0 / 64.0)
                nc.vector.scalar_tensor_tensor(
                    out=cqtl[:, lc, :], in0=ps, scalar=1.0 / 64.0,
                    in1=cqth[:, lc, :], op0=Mult, op1=Sub)

            # ---------- CqWqk^T per head (3-term), psC rotations ----------
            def emit_cqw(h):
                ch = cqwp.tile([128, 2, NQ], f8, tag="ch", name=f"cqwh{h}")
                cl = cqwp.tile([128, 2, NQ], f8, tag="cl", name=f"cqwl{h}")
                for mc in range(2):
                    psC = psCp.tile([128, 512], f32, tag="c",
                                    name=f"cqw{h}_{mc}")
                    msl = slice(h * LKV + mc * 128, h * LKV + (mc + 1) * 128)
                    k = 0
                    for wt, qt in ((wqkh, cqth), (wqkh, cqtl), (wqkl, cqth)):
                        for t2 in range(2):
                            nc.tensor.matmul(
                                psC, wt[:, 2 * t2:2 * t2 + 2, msl],
                                qt[:, 2 * t2:2 * t2 + 2, :],
                                start=(k == 0), stop=(k == 5), perf_mode=DR)
                            k += 1
                    nc.vector.tensor_scalar_mul(ch[:, mc, :], psC, 1.0 / 16.0)
                    nc.vector.scalar_tensor_tensor(
                        out=cl[:, mc, :], in0=psC, scalar=1.0 / 16.0,
                        in1=ch[:, mc, :], op0=Mult, op1=Sub)
                return ch, cl

            # ---------- phase 2: C_kv^T per kslice (3-term) ----------
            def emit_ckv_slice(ks):
                th, tl = kt_tiles[ks]
                for lkc in range(2):
                    ps = psB[:, lkc, 0:256]
                    k = 0
                    for wt, kt in ((wkvdh, th), (wkvdh, tl), (wkvdl, th)):
                        for j in range(EPAIR):
                            nc.tensor.matmul(
                                ps, wt[:, 2 * j:2 * j + 2,
                                       lkc * 128:(lkc + 1) * 128],
                                kt[:, 2 * j:2 * j + 2, :],
                                start=(k == 0), stop=(k == 3 * EPAIR - 1),
                                perf_mode=DR)
                            k += 1
                    nc.vector.tensor_scalar_mul(
                        ckvth[ks][:, lkc, :], ps, 1.0 / 64.0)
                    nc.vector.scalar_tensor_tensor(
                        out=ckvtl[ks][:, lkc, :], in0=ps, scalar=1.0 / 64.0,
                        in1=ckvth[ks][:, lkc, :], op0=Mult, op1=Sub)

            def emit_transpose_slice(ks):
                ptr = psCp.tile([128, 2, 2, 2, 128, 2], f8, tag="c",
                                name=f"ptr{ks}")
                for hl, src in enumerate((ckvth[ks], ckvtl[ks])):
                    for kc2 in range(2):
                        for lkc in range(2):
                            nc.tensor.transpose(
                                ptr[:, hl, kc2, lkc, :, 0],
                                src[:, lkc, kc2 * 128:(kc2 + 1) * 128], idn8)
                for hl, dst in enumerate((ckv8h[ks], ckv8l[ks])):
                    nc.vector.tensor_copy(
                        dst, ptr[:, hl, :, :, :, 0]
                        .rearrange("p k l i -> p k (l i)"))

            cqw_tiles = [None] * H
            cqw_tiles[0] = emit_cqw(0)
            for ks in range(KSL):
                emit_ckv_slice(ks)
                emit_transpose_slice(ks)

            # ---------- phase 3: per-head attention ----------
            if do_heads:
                def emit_tail(h):
                    # up-proj (fp16) + normalize + store for head h
                    olt16 = hp.tile([128, 2, NQ], f16, tag="olt")
                    nc.vector.tensor_copy(olt16, psB)
                    invd = hp.tile([128, NCK], f32, tag="invd")
                    nc.vector.reciprocal(invd, psD)
                    ot16 = hp.tile([128, NCK, HD], f16, tag="ot")
                    for nk in range(NCK):
                        psC = psCp.tile([128, 512], f32, tag="c",
                                        name=f"up{h}_{nk}")
                        for lc in range(2):
                            nc.tensor.matmul(
                                psC[:, 0:256],
                                olt16[:, lc, nk * 128:(nk + 1) * 128],
                                wvu16[:, lc, h * HD:(h + 1) * HD],
                                start=(lc == 0), stop=(lc == 1))
                        nc.vector.tensor_scalar_mul(
                            ot16[:, nk, :], psC[:, 0:256], invd[:, nk:nk + 1])
                    nc.gpsimd.dma_start(
                        out=OUT.rearrange("(c p) e -> p c e", p=128)
                        [:, :, h * HD:(h + 1) * HD],
                        in_=ot16)

                for h in range(H):
                    ch, cl = cqw_tiles[h]
                    pts = {}
                    for j in range(KPAIR + 1):
                        if j < KPAIR:
                            # S^T pair j: 3-term into alternating ring tiles
                            ps = psA0 if j % 2 == 0 else psA1
                            for kc in (2 * j, 2 * j + 1):
                                at_h = ckvth[kc // 2]
                                at_l = ckvtl[kc // 2]
                                ksl = slice((kc % 2) * 128,
                                            (kc % 2 + 1) * 128)
                                for ti, (a, bq) in enumerate(
                                        ((at_h, ch), (at_h, cl),
                                         (at_l, ch))):
                                    nc.tensor.matmul(
                                        ps[:, kc % 2, :], a[:, :, ksl], bq,
                                        start=(ti == 0), stop=(ti == 2),
                                        perf_mode=DR)
                            # bias -3.5 keeps exp within fp8e4m3 range (max
                            # 240; max score 7.6); softmax shift-invariant
                            pt8 = ptp.tile([128, 2, NQ], f8, tag="pt")
                            nc.scalar.activation(
                                out=pt8, in_=ps,
                                func=Exp, scale=1.0 / 16.0, bias=bias2[:, :])
                            pts[j] = pt8
                        if j == 1:
                            if h + 1 < H:
                                cqw_tiles[h + 1] = emit_cqw(h + 1)
                            if h > 0:
                                emit_tail(h - 1)
                        if j > 0:
                            jm = j - 1
                            pt8 = pts.pop(jm)
                            for lc in range(2):
                                for ti, cv in enumerate(
                                        (ckv8h[jm], ckv8l[jm])):
                                    nc.tensor.matmul(
                                        psB[:, lc, :],
                                        cv[:, :, lc * 128:(lc + 1) * 128],
                                        pt8,
                                        start=(jm == 0 and ti == 0),
                                        stop=(jm == KPAIR - 1 and ti == 1),
                                        perf_mode=DR)
                            for nk in range(NCK):
                                # start=True clears the whole bank: only the
                                # first group may use it (clears all slots)
                                nc.tensor.matmul(
                                    psD[:, nk:nk + 1],
                                    pt8[:, :, nk * 128:(nk + 1) * 128],
                                    ones8,
                                    start=(jm == 0 and nk == 0),
                                    stop=(jm == KPAIR - 1),
                                    perf_mode=DR, skip_group_check=True)
                emit_tail(H - 1)
            else:
                dummy = persist.tile([128, NCK, E], f16)
                nc.vector.memset(dummy, 0.5)
                nc.sync.dma_start(
                    out=OUT.rearrange("(c p) e -> p c e", p=128),
                    in_=dummy)

            if loop_ctx is not None:
                loop_ctx.__exit__(None, None, None)

    nc.finalize()
    return nc


def get_nc(iters=1, stop_after="full", fake_ckv=False):
    key = (iters, stop_after)
    if key not in _cache:
        _cache[key] = build_nc(iters, stop_after)
    return _cache[key]


F8 = ml_dtypes.float8_e4m3


def _hilo(x):
    hi = np.ascontiguousarray(x).astype(F8)
    lo = np.ascontiguousarray(x - hi.astype(np.float32)).astype(F8)
    return hi, lo


def make_in_maps(Q, K, Wq_d, W_qk, Wkv_d, Wv_u):
    Q = np.asarray(Q, dtype=np.float32)
    K = np.asarray(K, dtype=np.float32)
    wqdh, wqdl = _hilo(64.0 * np.asarray(Wq_d, np.float32))
    wqkh, wqkl = _hilo(16.0 * np.asarray(W_qk, np.float32))
    wkvdh, wkvdl = _hilo(64.0 * np.asarray(Wkv_d, np.float32))
    weights = {
        "WQDH": wqdh, "WQDL": wqdl,
        "WQKH": wqkh, "WQKL": wqkl,
        "WKVDH": wkvdh, "WKVDL": wkvdl,
        "WVU": np.ascontiguousarray(
            np.asarray(Wv_u, np.float32)).astype(np.float16),
        "IDN": np.eye(128, dtype=np.float32).astype(F8),
    }
    kts = [_hilo(K[b].T) for b in range(B)]
    qts = [_hilo(Q[b].T) for b in range(B)]
    in_maps = []
    for c in range(NCORES):
        b, q = divmod(c, 4)
        m = dict(weights)
        m["KTH"], m["KTL"] = kts[b]
        m["QTH"] = np.ascontiguousarray(qts[b][0][:, q * NQ:(q + 1) * NQ])
        m["QTL"] = np.ascontiguousarray(qts[b][1][:, q * NQ:(q + 1) * NQ])
        in_maps.append(m)
    return in_maps


def kernel(Q, K, Wq_d, W_qk, Wkv_d, Wv_u):
    from concourse.bass_utils import run_bass_kernel_spmd

    nc = get_nc(1)
    in_maps = make_in_maps(Q, K, Wq_d, W_qk, Wkv_d, Wv_u)
    res = run_bass_kernel_spmd(nc, in_maps, core_ids=list(range(NCORES)))
    out = np.empty((B, N, E), dtype=np.float32)
    for c in range(NCORES):
        b, q = divmod(c, 4)
        out[b, q * NQ:(q + 1) * NQ, :] = \
            res.results[c]["OUT"].astype(np.float32)
    return out


# revision 22
# speedup vs baseline: 4.3056x; 4.3056x over previous
"""MLA (absorbed-weight multi-head latent attention) TRN2 Bass kernel.

Problem: B=2, N=NKV=2048, E=4096, H=16, HD=256, LQ=512, LKV=256.
  C_q  = Q @ Wq_d                 [B,N,LQ]
  C_kv = K @ Wkv_d                [B,Nkv,LKV]
  CqWqk = (C_q @ W_qk)            [B,N,H,LKV]
  scores = einsum('bnhl,bkl->bhnk', CqWqk, C_kv) / sqrt(LKV)
  attn = softmax(scores, -1)
  V_up = (C_kv @ Wv_u)            [B,Nkv,H,HD]
  out  = einsum('bhnk,bkhd->bnhd', attn, V_up) -> [B,N,E]

Sharding: 8 cores = (batch b in 0..1) x (query quarter q in 0..3).
Each core handles n-rows [q*512,(q+1)*512) of batch b for ALL heads.

All matmuls run in float32r (TF32-like, ~2^-11 rel rounding, full PE rate).
Host passes Q^T and K^T slices so the device needs no transposes:
  C_qT   [LQ, n]  = lhsT Wq_d   @ rhs Q^T      (contract E)
  C_kvT  [LKV, k] = lhsT Wkv_d  @ rhs K^T      (contract E)
  CqWqkT [LKV, n] = lhsT W_qk_h @ rhs C_qT     (contract LQ)
  Vup_h  [k, HD]  = lhsT C_kvT  @ rhs Wv_u_h   (contract LKV)
  S^T    [k, n]   = lhsT C_kvT  @ rhs CqWqkT_h (contract LKV)
  P^T    = exp(S^T / 16)  (no max-subtraction: |S| <= ~6, fp32-safe)
  out    [n, HD+] = lhsT P^T    @ rhs [Vup_h | 1 1]  (contract k)
  out[:, :256] /= out[:, 256]  (ones-column row-sum denominator)
"""
import numpy as np

B, N, NKV, E, H = 2, 2048, 2048, 4096, 16
HD, LQ, LKV = 256, 512, 256
NCORES = 8
NQ = N // 4          # 512 query rows per core
ECH = E // 128       # 32 e-chunks
KCH = NKV // 128     # 16 k-chunks
NCK = NQ // 128      # 4 n-chunks per core

_cache = {}


def build_nc(iters=1, stop_after="full", fake_ckv=False):
    import concourse.bass as bass
    from concourse import bacc
    import concourse.mybir as mybir
    import concourse.tile as tile

    dt = mybir.dt
    f32r = dt.float32r
    f32 = dt.float32
    do_proj = stop_after in ("proj", "scores", "full")
    do_headmm = stop_after in ("scores", "full")
    do_scores = stop_after == "full"

    nc = bacc.Bacc(None, target_bir_lowering=False)
    QT = nc.dram_tensor("QT", [E, NQ], f32r, kind="ExternalInput")
    KT = nc.dram_tensor("KT", [E, NKV], f32r, kind="ExternalInput")
    WQD = nc.dram_tensor("WQD", [E, LQ], f32r, kind="ExternalInput")
    WQK = nc.dram_tensor("WQK", [LQ, H * LKV], f32r, kind="ExternalInput")
    WKVD = nc.dram_tensor("WKVD", [E, LKV], f32r, kind="ExternalInput")
    WVU = nc.dram_tensor("WVU", [LKV, H * HD], f32r, kind="ExternalInput")
    ONES = nc.dram_tensor("ONES", [128, 32], f32r, kind="ExternalInput")
    OUT = nc.dram_tensor("OUT", [NQ, E], f32, kind="ExternalOutput")

    Exp = mybir.ActivationFunctionType.Exp

    with tile.TileContext(nc) as tc:
        with tc.tile_pool(name="persist", bufs=1) as persist, \
             tc.tile_pool(name="psumA", bufs=1, space="PSUM") as psA, \
             tc.tile_pool(name="psumB", bufs=2, space="PSUM") as psB:
            loop_ctx = tc.For_i(0, iters, 1,
                                hint_engines=(mybir.EngineType.PE,)) \
                if iters > 1 else None
            if loop_ctx is not None:
                loop_ctx.__enter__()

            cqt = persist.tile([128, 4, NQ], f32r)       # C_qT  [LQ, n]
            ckvt = persist.tile([128, 2, NKV], f32r)     # C_kvT [LKV, k]

            # ---------- phase 1: C_qT (sliced loads), phase 2: C_kvT ----------
            with tc.tile_pool(name="ph1", bufs=1) as ph1, \
                 tc.tile_pool(name="ktp", bufs=3) as ktp:
                qt = ph1.tile([128, ECH, NQ], f32r)
                wqd = ph1.tile([128, ECH, LQ], f32r)
                nc.sync.dma_start(out=qt, in_=QT.rearrange("(c p) n -> p c n", p=128))
                nc.sync.dma_start(out=wqd, in_=WQD.rearrange("(c p) l -> p c l", p=128))
                if do_proj:
                    # C_qT: 4 x [128, 512], contract E
                    for lc in range(4):
                        ps = psB.tile([128, 512], f32, tag="sw")
                        for ec in range(ECH):
                            nc.tensor.matmul(ps, wqd[:, ec, lc * 128:(lc + 1) * 128],
                                             qt[:, ec, :],
                                             start=(ec == 0), stop=(ec == ECH - 1))
                        nc.vector.tensor_copy(cqt[:, lc, :], ps)
                else:
                    nc.vector.tensor_copy(cqt[:, 0, :], qt[:, 0, :])

                # C_kvT: 8 accumulators [2 lkc x 4 ktile] over streamed KT
                if fake_ckv:
                    nc.sync.dma_start(
                        out=ckvt,
                        in_=KT[0:256, :].rearrange("(l p) n -> p l n", p=128))
                if do_proj and not fake_ckv:
                    accs = [psA.tile([128, 512], f32, tag=t, name=f"acc_{t}")
                            for t in ("o0", "o1", "o2", "o3")]
                    accs += [psB.tile([128, 512], f32, tag=t, name=f"acc2_{i}")
                             for i, t in enumerate(("v", "v", "sw", "sw"))]
                for ec in range(0 if not fake_ckv else ECH, ECH):
                    ktt = ktp.tile([128, NKV], f32r, tag="kt")
                    nc.sync.dma_start(out=ktt, in_=KT[ec * 128:(ec + 1) * 128, :])
                    if ec % 4 == 0:
                        wkvd_t = ktp.tile([128, 4, LKV], f32r, tag="wkvd")
                        nc.sync.dma_start(
                            out=wkvd_t,
                            in_=WKVD[ec * 128:(ec + 4) * 128, :]
                            .rearrange("(c p) l -> p c l", p=128))
                    if do_proj:
                        for lc in range(2):
                            for nt in range(4):
                                nc.tensor.matmul(
                                    accs[lc * 4 + nt],
                                    wkvd_t[:, ec % 4, lc * 128:(lc + 1) * 128],
                                    ktt[:, nt * 512:(nt + 1) * 512],
                                    start=(ec == 0), stop=(ec == ECH - 1))
                    else:
                        if ec == 0:
                            nc.vector.tensor_copy(ckvt[:, 0, 0:NKV], ktt)
                if do_proj and not fake_ckv:
                    for lc in range(2):
                        for nt in range(4):
                            dst = ckvt[:, lc, nt * 512:(nt + 1) * 512]
                            if nt % 2 == 0:
                                nc.vector.tensor_copy(dst, accs[lc * 4 + nt])
                            else:
                                nc.scalar.copy(dst, accs[lc * 4 + nt])

            # ---------- phase 3: per-head attention ----------
            with tc.tile_pool(name="head", bufs=2) as hp, \
                 tc.tile_pool(name="ptp", bufs=3) as ptp:
                for h in range(H):
                    wqk_h = hp.tile([128, 4, LKV], f32r, tag="wqk")
                    nc.sync.dma_start(
                        out=wqk_h,
                        in_=WQK[:, h * LKV:(h + 1) * LKV]
                        .rearrange("(c p) l -> p c l", p=128))
                    wvu_h = hp.tile([128, 2, HD], f32r, tag="wvu")
                    nc.sync.dma_start(
                        out=wvu_h,
                        in_=WVU[:, h * HD:(h + 1) * HD]
                        .rearrange("(c p) d -> p c d", p=128))

                    if do_headmm:
                        vup = hp.tile([128, KCH, 258], f32r, tag="vup")
                        nc.sync.dma_start(
                            out=vup[:, :, 256:258],
                            in_=ONES.rearrange("p (g c) -> p g c", c=2))
                        # CqWqkT_h [2 x 128, n=512], contract LQ
                        cqwqk = hp.tile([128, 2, NQ], f32r, tag="cqwqk")
                        for lkc in range(2):
                            ps = psB.tile([128, 512], f32, tag="sw")
                            for lc in range(4):
                                nc.tensor.matmul(
                                    ps, wqk_h[:, lc, lkc * 128:(lkc + 1) * 128],
                                    cqt[:, lc, :], start=(lc == 0), stop=(lc == 3))
                            nc.scalar.copy(cqwqk[:, lkc, :], ps)

                        # V_up rows for this head, [k, 256] per k-chunk
                        for kc in range(KCH):
                            psv = psB.tile([128, 256], f32, tag="v")
                            for lkc in range(2):
                                nc.tensor.matmul(
                                    psv, ckvt[:, lkc, kc * 128:(kc + 1) * 128],
                                    wvu_h[:, lkc, :],
                                    start=(lkc == 0), stop=(lkc == 1))
                            nc.vector.tensor_copy(vup[:, kc, 0:256], psv)

                    if do_scores:
                        # scores^T -> exp -> PV accumulate
                        pso = [psA.tile([128, 258], f32, tag=f"o{i}",
                                        name=f"pso{i}") for i in range(NCK)]
                        for kc in range(KCH):
                            pss = psB.tile([128, 512], f32, tag="sw")
                            for lkc in range(2):
                                nc.tensor.matmul(
                                    pss, ckvt[:, lkc, kc * 128:(kc + 1) * 128],
                                    cqwqk[:, lkc, :],
                                    start=(lkc == 0), stop=(lkc == 1))
                            pt = ptp.tile([128, NQ], f32r, tag="pt")
                            nc.scalar.activation(out=pt, in_=pss, func=Exp,
                                                 scale=1.0 / 16.0)
                            for nk in range(NCK):
                                nc.tensor.matmul(
                                    pso[nk], pt[:, nk * 128:(nk + 1) * 128],
                                    vup[:, kc, :],
                                    start=(kc == 0), stop=(kc == KCH - 1))

                        # normalize + store (one batched DMA per head)
                        ot = ptp.tile([128, NCK, HD], f32, tag="ot")
                        for nk in range(NCK):
                            den = hp.tile([128, 1], f32, tag="den")
                            nc.vector.reciprocal(den, pso[nk][:, 256:257])
                            nc.vector.tensor_scalar_mul(ot[:, nk, :],
                                                        pso[nk][:, 0:256], den)
                        nc.sync.dma_start(
                            out=OUT.rearrange("(c p) e -> p c e", p=128)
                            [:, :, h * HD:(h + 1) * HD],
                            in_=ot)
                    else:
                        dummy = ptp.tile([128, NCK, HD], f32, tag="ot")
                        nc.vector.memset(dummy, 0.5)
                        nc.sync.dma_start(
                            out=OUT.rearrange("(c p) e -> p c e", p=128)
                            [:, :, h * HD:(h + 1) * HD],
                            in_=dummy)

            if loop_ctx is not None:
                loop_ctx.__exit__(None, None, None)

    nc.finalize()
    return nc


def get_nc(iters=1, stop_after="full", fake_ckv=False):
    key = (iters, stop_after, fake_ckv)
    if key not in _cache:
        _cache[key] = build_nc(iters, stop_after, fake_ckv)
    return _cache[key]


def make_in_maps(Q, K, Wq_d, W_qk, Wkv_d, Wv_u):
    Q = np.asarray(Q, dtype=np.float32)
    K = np.asarray(K, dtype=np.float32)
    ones = np.ones((128, 32), dtype=np.float32)
    weights = {
        "WQD": np.ascontiguousarray(np.asarray(Wq_d, dtype=np.float32)),
        "WQK": np.ascontiguousarray(np.asarray(W_qk, dtype=np.float32)),
        "WKVD": np.ascontiguousarray(np.asarray(Wkv_d, dtype=np.float32)),
        "WVU": np.ascontiguousarray(np.asarray(Wv_u, dtype=np.float32)),
        "ONES": ones,
    }
    kts = [np.ascontiguousarray(K[b].T) for b in range(B)]
    qts = [np.ascontiguousarray(Q[b].T) for b in range(B)]
    in_maps = []
    for c in range(NCORES):
        b, q = divmod(c, 4)
        m = dict(weights)
        m["KT"] = kts[b]
        m["QT"] = np.ascontiguousarray(qts[b][:, q * NQ:(q + 1) * NQ])
        in_maps.append(m)
    return in_maps


def kernel(Q, K, Wq_d, W_qk, Wkv_d, Wv_u):
    from concourse.bass_utils import run_bass_kernel_spmd

    nc = get_nc(1)
    in_maps = make_in_maps(Q, K, Wq_d, W_qk, Wkv_d, Wv_u)
    res = run_bass_kernel_spmd(nc, in_maps, core_ids=list(range(NCORES)))
    out = np.empty((B, N, E), dtype=np.float32)
    for c in range(NCORES):
        b, q = divmod(c, 4)
        out[b, q * NQ:(q + 1) * NQ, :] = res.results[c]["OUT"]
    return out

